# revision 20
# baseline (speedup 1.0000x reference)
"""Multi-head self-attention (N=2048, DIM=1024, NH=16, DK=64) on 8 trn2 cores.

Head-parallel sharding: core c computes heads 2c and 2c+1.
Per core: Q/K/V projections for its 128 head-dims, scores in [m, n] layout
(row-packed K=64 matmuls, both heads concurrent in the PE array), exp on ACT,
att = [V | 1]^T @ E accumulated over m-tiles (ones column yields the softmax
denominator as row 64).  Output stays in [d, n] layout: divide rows 0..63 by
row 64 (fast-approx recip on DVE, partition-broadcast on Pool, multiply on
DVE) and DMA out as [J, N]; the host transposes while unsharding.
x is DMA'd as contiguous 256KB pieces in need-order across the three DMA
rings; lead-in projections are gated per x-piece.  AV matmuls run through a
lag-8 FIFO so pass-0's projection burst overlaps the ACT-bound later passes.
"""

import sys
from contextlib import ExitStack

import numpy as np

for _p in ("/opt/trn_rl_repo", "/root/.axon_site/_ro/trn_rl_repo"):
    if _p not in sys.path:
        sys.path.insert(0, _p)

import ml_dtypes  # noqa: E402

import concourse.bass as bass  # noqa: E402
import concourse.bacc as bacc  # noqa: E402
import concourse.mybir as mybir  # noqa: E402
import concourse.tile as tile  # noqa: E402
from concourse.bass import ds, ts  # noqa: E402
from concourse.bass_utils import run_bass_kernel_spmd  # noqa: E402

N = 2048
DIM = 1024
NH = 16
DK = 64
NCORES = 8
J = 128          # head dims per core (2 heads x 64)
KT = DIM // 128  # 8 contraction tiles
MT = N // 128    # 16 m-tiles
P = 128
NPC = 16         # x DMA pieces (2 k-tiles each)

F32 = mybir.dt.float32
BF16 = mybir.dt.bfloat16
EXP = mybir.ActivationFunctionType.Exp

_NC_CACHE = {}


def build_nc():
    nc = bacc.Bacc("TRN2", target_bir_lowering=False, debug=False)

    # x host-packed as contiguous pieces: piece (q*4 + kk) holds k-tiles
    # 2kk..2kk+1 of n-quarter q; element (pc, p, kl*512 + n) =
    # x[q*512+n, (2*kk+kl)*128+p]
    x_d = nc.dram_tensor("xt", [NPC, P, 1024], BF16, kind="ExternalInput")
    # weights host-packed: element (p, k*J+j) = W^T[k*128+p, j]
    wq_d = nc.dram_tensor("wqt", [P, KT * J], BF16, kind="ExternalInput")
    wk_d = nc.dram_tensor("wkt", [P, KT * J], BF16, kind="ExternalInput")
    wv_d = nc.dram_tensor("wvt", [P, KT * J], BF16, kind="ExternalInput")
    # output in [j, n] layout; host transposes while unsharding
    out_d = nc.dram_tensor("out", [J, N], F32, kind="ExternalOutput")

    with tile.TileContext(nc) as tc, ExitStack() as ctx:
        pers = ctx.enter_context(tc.tile_pool(name="pers", bufs=1))
        etp = ctx.enter_context(tc.tile_pool(name="et", bufs=19))
        vnp = ctx.enter_context(tc.tile_pool(name="vn", bufs=6))
        rcp = ctx.enter_context(tc.tile_pool(name="rc", bufs=4))
        rbp = ctx.enter_context(tc.tile_pool(name="rb", bufs=4))
        obp = ctx.enter_context(tc.tile_pool(name="ob", bufs=4))
        stp = ctx.enter_context(
            tc.tile_pool(name="stp", bufs=2, space=bass.MemorySpace.PSUM)
        )
        opp = ctx.enter_context(
            tc.tile_pool(name="opp", bufs=2, space=bass.MemorySpace.PSUM)
        )
        pjp = ctx.enter_context(
            tc.tile_pool(name="pjp", bufs=1, space=bass.MemorySpace.PSUM)
        )
        ttp = ctx.enter_context(
            tc.tile_pool(name="ttp", bufs=1, space=bass.MemorySpace.PSUM)
        )

        # ---- persistent SBUF tensors
        x_sb = pers.tile([P, 4, KT, 512], BF16, tag="x")
        wq_sb = pers.tile([P, KT, J], BF16, tag="wq")
        wk_sb = pers.tile([P, KT, J], BF16, tag="wk")
        wv_sb = pers.tile([P, KT, J], BF16, tag="wv")
        qt_sb = pers.tile([P, N], BF16, tag="qt")
        kt_sb = pers.tile([P, N], BF16, tag="kt")
        vt_sb = pers.tile([P, N], BF16, tag="vt")
        vp_sb = pers.tile([P, MT, 2, DK + 1], BF16, tag="vp")
        wu_i = pers.tile([1, 1], F32, tag="wui")
        wu_o = pers.tile([1, 1], F32, tag="wuo")
        wrm = pers.tile([P, 512], BF16, tag="wrm")

        # ---- ACT exp-table warmup + warmup-matmul source; memsets on DVE so
        # the DMA-issuing queues stay free
        nc.vector.memset(wu_i[:, :], 0.0)
        nc.scalar.activation(wu_o[:, :], wu_i[:, :], EXP)
        nc.vector.memset(wrm[:, :], 0.0)
        # ones columns for the attention matmuls (denominator trick)
        nc.vector.memset(vp_sb[:, :, :, :], 1.0)

        # ---- input DMAs, need-order per ring.  piece (q, kk) -> x k-tiles
        # 2kk..2kk+1 of quarter q.
        def xpiece(eng, q, kk):
            eng.dma_start(
                x_sb[:, q, 2 * kk:2 * kk + 2, :], x_d[q * 4 + kk]
            )

        # Rings are FIFO: order each ring's transfers by need so the lead-in
        # data (q0 + wk + wq) finishes first instead of sharing bandwidth
        # with everything queued behind it.
        # sync ring:   q0k01 q0k45 | q1k01 q1k45 | q2k01 q2k45
        # scalar ring: q0k23 q0k67 | q1k23 q1k67 | q2k23 q2k67
        # gpsimd ring: wk wq wv | q3 (all four pieces)
        # weights ride the FAST rings first (the gpsimd ring gets the least
        # bandwidth); then q0 pieces, q1, q2; wv + q3 go on gpsimd
        nc.sync.dma_start(wk_sb[:, :, :], wk_d[:, :])
        nc.scalar.dma_start(wq_sb[:, :, :], wq_d[:, :])
        nc.gpsimd.dma_start(wv_sb[:, :, :], wv_d[:, :])
        xpiece(nc.sync, 0, 0)
        xpiece(nc.scalar, 0, 1)
        xpiece(nc.sync, 0, 2)
        xpiece(nc.scalar, 0, 3)
        xpiece(nc.sync, 1, 0)
        xpiece(nc.scalar, 1, 1)
        xpiece(nc.sync, 1, 2)
        xpiece(nc.scalar, 1, 3)
        xpiece(nc.sync, 2, 0)
        xpiece(nc.scalar, 2, 1)
        xpiece(nc.sync, 2, 2)
        xpiece(nc.scalar, 2, 3)
        for kk in range(4):
            xpiece(nc.gpsimd, 3, kk)

        # warm the PE (p-state ramp) with junk matmuls while DMA is in flight
        wps = stp.tile([P, 512], F32, tag="st", name="warm_ps")
        for r in range(6):
            nc.tensor.matmul(
                wps[:, :], wrm[:, 0:P], wrm[:, :],
                start=(r == 0), stop=(r == 5),
            )

        _pj_alt = [0]
        _pj_live = {}

        def project_half(dst_sb, w_sb, n0, half):
            """Half a projection chunk (k-tiles 4*half..4*half+3); the second
            half finishes the accumulation and copies PSUM -> SBUF.  Split so
            the PE burst per unit stays under the exp cadence."""
            q = n0 // 512
            key = (id(w_sb), n0)
            if half == 0:
                pool, tg = ((pjp, "pj"), (ttp, "tt"))[_pj_alt[0] % 2]
                _pj_alt[0] += 1
                _pj_live[key] = pool.tile(
                    [P, 512], F32, tag=tg, name=f"pj_{key[0]}_{n0}"
                )
            ps = _pj_live[key]
            for k in range(4 * half, 4 * half + 4):
                nc.tensor.matmul(
                    ps[:, :],
                    w_sb[:, k, :],
                    x_sb[:, q, k, :],
                    start=(k == 0),
                    stop=(k == KT - 1),
                )
            if half == 1:
                nc.vector.tensor_copy(dst_sb[:, ds(n0, 512)], ps[:, :])
                del _pj_live[key]

        def project(dst_sb, w_sb, n0):
            project_half(dst_sb, w_sb, n0, 0)
            project_half(dst_sb, w_sb, n0, 1)

        def vprep(i):
            """Build V' tiles for m-tile i: transpose Vt block, split heads.
            Column 0 of each V' stays all-ones (denominator row lands in
            PSUM partition 0, where the custom-DVE recip reads reliably)."""
            vn = vnp.tile([P, P], BF16, tag="vn", name=f"vn{i}")
            nc.sync.dma_start_transpose(vn[:, :], vt_sb[:, ts(i, P)])
            nc.gpsimd.tensor_copy(vp_sb[:, i, 0, 0:DK], vn[:, 0:DK])
            nc.gpsimd.tensor_copy(vp_sb[:, i, 1, 0:DK], vn[:, DK:2 * DK])

        def scores_exp(i, p):
            """Scores for both heads (row-packed, concurrent) + exp; pass p."""
            n0 = p * 512
            st = stp.tile([P, 1024], F32, tag="st", name=f"st{p}_{i}")
            # h0 in rows 0-63 of the PE array, h1 in rows 64-127 (concurrent)
            nc.tensor.matmul(
                st[:, 0:512],
                kt_sb[0:DK, ts(i, P)],
                qt_sb[0:DK, ds(n0, 512)],
                start=True, stop=True,
                tile_position=(0, 0),
            )
            nc.tensor.matmul(
                st[:, 512:1024],
                kt_sb[DK:2 * DK, ts(i, P)],
                qt_sb[DK:2 * DK, ds(n0, 512)],
                start=True, stop=True,
                tile_position=(64, 0),
            )
            et = etp.tile([P, 1024], BF16, tag="et", name=f"et{p}_{i}")
            nc.scalar.activation(et[:, :], st[:, :], EXP)
            return et

        def att_emit(i, o_ps, et):
            for h in range(2):
                nc.tensor.matmul(
                    o_ps[h][:, :],
                    vp_sb[:, i, h, :],
                    et[:, ds(h * 512, 512)],
                    start=(i == 0),
                    stop=(i == MT - 1),
                )

        def fin(p, o_ps, direct=False):
            """Normalize O' rows 0..63 by the denominator row 64, DMA out in
            [j, n] layout.  A fast staging copy frees the PSUM accumulator
            banks immediately (the next pass's AV start reuses them); the
            divide chain (fast-approx recip on DVE at partition 0,
            partition-broadcast on Pool, multiply on DVE) runs from SBUF off
            the critical path.  direct=True (last pass) skips the staging
            copy and reads PSUM straight for a shorter tail."""
            if direct:
                src = o_ps
            else:
                src = [None, None]
                for h in range(2):
                    src[h] = obp.tile(
                        [DK + 1, 512], F32, tag="osb", name=f"osb{p}_{h}"
                    )
                    nc.vector.tensor_copy(src[h][:, :], o_ps[h][:, :])
            for h in range(2):
                dsb = rcp.tile([1, 512], F32, tag="dsb", name=f"dsb{p}_{h}")
                nc.vector.tensor_copy(dsb[:, :], src[h][DK:DK + 1, :])
                rc = rcp.tile([1, 512], F32, tag="rc", name=f"rc{p}_{h}")
                nc.vector.reciprocal_approx_fast(out=rc[:, :], in_=dsb[:, :])
                rb = rbp.tile([DK, 512], F32, tag="rb", name=f"rb{p}_{h}")
                nc.gpsimd.partition_broadcast(rb[:, :], rc[:, :])
                ob = obp.tile([DK, 512], F32, tag="ob", name=f"ob{p}_{h}")
                nc.vector.tensor_tensor(
                    ob[:, :], src[h][0:DK, :], rb[:, :],
                    op=mybir.AluOpType.mult,
                )
                # split across both DMA rings so the transfers overlap
                nc.sync.dma_start(
                    out_d[ds(h * DK, DK), ds(p * 512, 256)], ob[:, 0:256]
                )
                nc.gpsimd.dma_start(
                    out_d[ds(h * DK, DK), ds(p * 512 + 256, 256)],
                    ob[:, 256:512],
                )

        # ---- lead-in projections: K and Q interleaved per k-tile so both
        # consume x pieces the moment they land; CASTs on different engines
        ps_k = pjp.tile([P, 512], F32, tag="pj", name="lead_k")
        ps_q = ttp.tile([P, 512], F32, tag="tt", name="lead_q")
        _pj_alt[0] = 2  # keep downstream pool alternation phase
        for k in range(KT):
            nc.tensor.matmul(ps_k[:, :], wk_sb[:, k, :], x_sb[:, 0, k, :],
                             start=(k == 0), stop=(k == KT - 1))
            nc.tensor.matmul(ps_q[:, :], wq_sb[:, k, :], x_sb[:, 0, k, :],
                             start=(k == 0), stop=(k == KT - 1))
        nc.vector.tensor_copy(kt_sb[:, 0:512], ps_k[:, :])
        nc.scalar.copy(qt_sb[:, 0:512], ps_q[:, :])

        # projection half-chunks + V'-preps spread across the passes so the
        # PE load per unit stays under the exp cadence
        inserts = {
            1: [("pk", 512, 0)],
            2: [("pk", 512, 1)],
            3: [("pv", 0, 0)],
            4: [("pk", 1024, 0)],
            5: [("pk", 1024, 1)],
            6: [("pv", 0, 1)],
            7: [("pk", 1536, 0)],
            8: [("pk", 1536, 1)],
            9: [("vp", 0, 0)],
            10: [("pv", 512, 0)],
            11: [("pv", 512, 1)],
            12: [("vp", 4, 0)],
            13: [("pq", 512, 0)],
            14: [("pq", 512, 1)],
            17: [("pv", 1024, 0)],
            19: [("pv", 1024, 1)],
            20: [("vp", 8, 0)],
            21: [("pv", 1536, 0)],
            23: [("pv", 1536, 1)],
            24: [("vp", 12, 0)],
            28: [("pq", 1024, 0)],
            29: [("pq", 1024, 1)],
            36: [("pq", 1536, 0)],
            37: [("pq", 1536, 1)],
        }

        def do_insert(kind, a, half):
            if kind == "pq":
                project_half(qt_sb, wq_sb, a, half)
            elif kind == "pk":
                project_half(kt_sb, wk_sb, a, half)
            elif kind == "pv":
                project_half(vt_sb, wv_sb, a, half)
            else:
                for ii in range(a, a + 4):
                    vprep(ii)

        # AV lag curve: no AVs during projection-heavy pass 0, then a gentle
        # catch-up sized so each pass's PE work matches the ACT exp cadence
        def cap_at(u):
            if u < 16:
                return 16
            if u < 32:
                return 16 - round((u - 15) * 2 / 16)
            if u < 48:
                return 14 - round((u - 31) * 6 / 16)
            return max(0, 8 - round((u - 47) * 8 / 16))

        o_ps_all = []
        av_fifo = []
        for u in range(4 * MT):
            p, i = divmod(u, MT)
            if i == 0:
                o_ps_all.append(
                    [opp.tile([DK + 1, 512], F32, tag="o", name=f"o{p}_{h}")
                     for h in range(2)]
                )
            et = scores_exp(i, p)
            av_fifo.append((i, p, et))
            for kind, a, half in inserts.get(u, []):
                do_insert(kind, a, half)
            while len(av_fifo) > cap_at(u):
                ai, ap_, aet = av_fifo.pop(0)
                att_emit(ai, o_ps_all[ap_], aet)
                if ai == MT - 1:
                    # pass ap_ fully accumulated: free its PSUM banks now
                    fin(ap_, o_ps_all[ap_], direct=(ap_ == 3))
        while av_fifo:
            ai, ap_, aet = av_fifo.pop(0)
            att_emit(ai, o_ps_all[ap_], aet)
            if ai == MT - 1:
                fin(ap_, o_ps_all[ap_], direct=(ap_ == 3))

    nc.finalize()
    return nc


def make_in_maps(x, Wq, Wk, Wv):
    x = np.asarray(x, dtype=np.float32)
    Wq = np.asarray(Wq, dtype=np.float32)
    Wk = np.asarray(Wk, dtype=np.float32)
    Wv = np.asarray(Wv, dtype=np.float32)

    bf16 = ml_dtypes.bfloat16
    scale = 1.0 / np.sqrt(DK)
    # [NPC, P, 1024]: element (q*4+kk, p, kl*512+n) = x[q*512+n, (2kk+kl)*128+p]
    xt = x.T.reshape(KT, P, 4, 512)            # [k, p, q, n]
    xt = xt.reshape(4, 2, P, 4, 512)           # [kk, kl, p, q, n]
    xt = xt.transpose(3, 0, 2, 1, 4)           # [q, kk, p, kl, n]
    xt = np.ascontiguousarray(xt.reshape(NPC, P, 1024)).astype(bf16)

    def pack_w(w_slice):
        # [DIM, J] -> [P, KT*J]: element (p, k*J+j) = W^T[k*P+p, j]
        wt = w_slice.T.reshape(KT, P, J).transpose(1, 0, 2).reshape(P, KT * J)
        return np.ascontiguousarray(wt).astype(bf16)

    in_maps = []
    for c in range(NCORES):
        sl = slice(c * J, (c + 1) * J)
        in_maps.append({
            "xt": xt,
            "wqt": pack_w(Wq[sl, :] * scale),
            "wkt": pack_w(Wk[sl, :]),
            "wvt": pack_w(Wv[sl, :]),
        })
    return in_maps


def assemble(res):
    # per-core result is [J, N]; transpose to [N, J] and concat heads
    out = np.concatenate(
        [np.asarray(res.results[c]["out"]).T for c in range(NCORES)], axis=1
    )
    return np.ascontiguousarray(out.astype(np.float32))


def kernel(x, rela, Wq, Wk, Wv):
    in_maps = make_in_maps(x, Wq, Wk, Wv)
    if "nc" not in _NC_CACHE:
        _NC_CACHE["nc"] = build_nc()
    res = run_bass_kernel_spmd(_NC_CACHE["nc"], in_maps, core_ids=list(range(NCORES)))
    return assemble(res)


if __name__ == "__main__":
    rng = np.random.default_rng(0)
    x = rng.standard_normal((N, DIM), dtype=np.float32)
    b = 1.0 / np.sqrt(DIM)
    Wq = rng.uniform(-b, b, (DIM, DIM)).astype(np.float32)
    Wk = rng.uniform(-b, b, (DIM, DIM)).astype(np.float32)
    Wv = rng.uniform(-b, b, (DIM, DIM)).astype(np.float32)
    out = kernel(x, np.zeros(1, np.float32), Wq, Wk, Wv)
    print(out.shape, out.dtype)
